# revision 46
# baseline (speedup 1.0000x reference)
"""TRN2 Bass kernel for nn_MetaBaseline (DN4-style local-descriptor kNN).

Reference computation (per batch b):
  q = normalize(input1[b].reshape(75*100, 640), axis=-1)       # query patches
  s = normalize(input2[b].reshape(2500, 640), axis=-1)         # support descs
  scores = q @ s.T                                             # [7500, 2500]
  per way group w (columns [500w, 500w+500)): top-k per row, mean over k,
  then sum over the 100 patches of each query -> out [75, 5].

Sharding: data-parallel over (b, query-quarter): 8 cores, each handles one
batch's quarter of queries (19 queries padded) with that batch's full
support replicated (per the sharding hint).

V3 architecture (fp8, balanced PE/DVE at ~50us each):
- Shard-time input prep (host, part of the sharding/replication step):
  support features are L2-normalized, scaled by 16 (fp8 e4m3 dynamic range),
  cast to fp8 and laid out pre-transposed in 5 chunk bands; queries are cast
  to fp8 and pre-transposed WITHOUT normalization - a positive per-row scale
  cannot change that row's top-k, so 1/(k*16*|q_p|) is folded into the
  host-built indicator matrix that the device uses for the final per-query
  summation. The dominant compute - the 3.07 GMAC/core similarity matmul,
  the 4.7M-element/core top-k scan, and all reductions - runs on device.
- Scores: per (m-tile, way): 2 DoubleRow fp8 matmuls (chunk pairs 01, 23)
  + 1 plain fp8 matmul (chunk 4) accumulate [128, 500] into one PSUM bank.
  (A padded 3rd DoubleRow pair measured strictly slower on HW: DR streams
  both k-tiles' columns, so the zero band costs real feed cycles.)
- Top-k: DVE max8 straight from PSUM -> bf16 mxs [128, 5*8] per m-tile.
  max8 has no DVE fast modes (and measures ~600ns from SBUF or PSUM
  alike), so the 75 x 500-element scan (~46us) is the hard DVE floor;
  the PE's 3-matmul unit (~650ns) makes the loop mutually saturated.
- Finale per m-tile (pass 4): bf16 matmul of the pre-scaled indicator
  with mxs accumulates [19, 40] in PSUM across m-tiles; epilogue reduces
  the first k of each 8 directly from PSUM and DMAs out [19, 5] fp32.
- DMA choreography: only the pass-0 critical set (~0.8MB) transfers
  immediately; the rest is issued later from idle engines (behind dummy
  delay ops) so it does not steal HBM bandwidth from the critical set.
"""
import os
from contextlib import ExitStack

import numpy as np
import ml_dtypes

import concourse.bass as bass  # noqa: F401
import concourse.mybir as mybir
import concourse.tile as tile
from concourse import bacc
from concourse.bass_utils import run_bass_kernel_spmd

# Problem geometry (hardcoded per contest rules)
B, Q, WAY, SHOT, H, W, C = 2, 75, 5, 5, 10, 10, 640
HW = H * W               # 100 patches per query / support image
NQ = 19                  # queries per core (4 cores x 19 = 76 >= 75)
MT = 15                  # patch M-tiles of 128 -> 1920 rows (1900 real)
PAD_P = MT * 128
NS = WAY * SHOT * HW     # 2500 support descriptors per batch
PAD_S = 2560             # padded support count (20 tiles of 128)
KC = 5                   # C chunks of 128 (640 = 5*128)
P = 128
NW = SHOT * HW           # 500 support descriptors per way group
N_CORES = 8
N_WARM = int(os.environ.get("N_WARM", "34"))

FP8 = ml_dtypes.float8_e4m3

_prog_cache: dict[int, object] = {}


def _build(k: int):
    """Build + compile the per-core SPMD program for neighbor_k == k."""
    assert 1 <= k <= 8, f"neighbor_k={k} not supported (need 1..8)"
    nc = bacc.Bacc("TRN2", target_bir_lowering=False, debug=False)
    f32 = mybir.dt.float32
    bf16 = mybir.dt.bfloat16
    fp8 = mybir.dt.float8e4
    DR = mybir.MatmulPerfMode.DoubleRow

    qT_d = nc.dram_tensor("qT", [P, KC * PAD_P], fp8, kind="ExternalInput").ap()
    sT_d = nc.dram_tensor("sT", [P, KC * PAD_S], fp8, kind="ExternalInput").ap()
    ind_d = nc.dram_tensor("ind", [P, MT * NQ], bf16, kind="ExternalInput").ap()
    # output is [way, query] (transposed): the finale matmul uses the tiny
    # [128, 5] reduced-topk tile as the stationary operand (cheap LDW) and
    # the indicator as the moving one; host gather transposes back
    out_d = nc.dram_tensor("out", [WAY, NQ], f32, kind="ExternalOutput").ap()

    with tile.TileContext(nc) as tc:
        with ExitStack() as ctx:
            const = ctx.enter_context(tc.tile_pool(name="const", bufs=1))
            big = ctx.enter_context(tc.tile_pool(name="big", bufs=1))
            mxp = ctx.enter_context(tc.tile_pool(name="mxp", bufs=MT))
            outp = ctx.enter_context(
                tc.tile_pool(name="outp", bufs=1, space="PSUM")
            )
            spp = ctx.enter_context(
                tc.tile_pool(name="spp", bufs=7, space="PSUM")
            )

            qT = big.tile([P, KC * PAD_P], fp8, name="qT")     # chunk bands
            sT = big.tile([P, KC * PAD_S], fp8, name="sT")     # chunk bands
            ind_sb = const.tile([P, MT * NQ], bf16, name="ind_sb")
            qT6 = qT.rearrange("p (c n) -> p c n", c=KC)
            sT6 = sT.rearrange("p (c n) -> p c n", c=KC)

            out_ps = outp.tile([WAY, NQ], f32)

            # ---- PE warmup (HAM clock ramp; no DMA deps) ----
            wtile = const.tile([P, P], fp8, name="wtile")
            nc.vector.memset(wtile, 1.0)
            for i in range(N_WARM):
                wps = spp.tile([P, NW], f32, tag="psc", name=f"w{i}")
                nc.tensor.matmul(wps[:, 0:P], wtile, wtile,
                                 start=True, stop=True)

            # ---- DMAs, issue-parallel across idle engines ----
            # The first (m,w) unit needs qT m-tiles 0-4 (all bands) and sT
            # way 0 only (~0.8MB); everything else streams under pass 0+.
            # ACT: qT in three m-tile thirds; Sync: sT way-slices in pass
            # order; GpSimd: indicator (needed only in pass 4).
            sT_dv = sT_d.rearrange("p (c n) -> p c n", c=KC)
            qT_dv = qT_d.rearrange("p (c n) -> p c n", c=KC)
            # immediate critical set (first unit needs sT way 0 + qT m0):
            # sT way-0 bands split sync/ACT, qT m-tiles 0-4 split ACT/GpSimd
            # immediate critical set (first unit needs sT way 0 + qT m0):
            # sT way-0 bands on Sync, qT m-tiles 0-4 split ACT/GpSimd
            nc.sync.dma_start(out=sT6[:, :, 0:NW], in_=sT_dv[:, :, 0:NW])
            MG = 5 * P  # m-tile third (5 tiles of 128 cols)
            HG = MG // 2
            nc.scalar.dma_start(out=qT6[:, :, 0:HG], in_=qT_dv[:, :, 0:HG])
            nc.gpsimd.dma_start(out=qT6[:, :, HG:MG], in_=qT_dv[:, :, HG:MG])
            # deferred behind idle-engine delay ops so their descriptors
            # don't compete with the critical set:
            # ACT: qT thirds 1-2 after dummy copies
            acts = const.tile([P, P], f32, name="acts")
            for g in (1, 2):
                for _ in range(14 if g == 1 else 6):
                    nc.scalar.copy(acts, wtile)
                nc.scalar.dma_start(
                    out=qT6[:, :, g * MG:(g + 1) * MG],
                    in_=qT_dv[:, :, g * MG:(g + 1) * MG])
            # GpSimd: sT ways 1-4 + indicator, spaced by memsets
            gscr = const.tile([P, 2048], fp8, name="gscr")
            for w in (1, 2, 3, 4):
                for _ in range(3 if w == 1 else 2):
                    nc.gpsimd.memset(gscr, 0.0)
                nc.gpsimd.dma_start(
                    out=sT6[:, :, w * NW:(w + 1) * NW],
                    in_=sT_dv[:, :, w * NW:(w + 1) * NW])
            nc.gpsimd.dma_start(out=ind_sb, in_=ind_d)

            # ---- main loop: way-outer, m-tile inner ----
            mxs = [None] * MT
            rxs = [None] * MT
            for w in range(WAY):
                for m in range(MT):
                    if w == 0:
                        mxs[m] = mxp.tile([P, WAY * 8], bf16, tag="mx",
                                          name=f"mx{m}")
                    psc = spp.tile([P, NW], f32, tag="psc",
                                   name=f"psc{m}_{w}")
                    for i in range(2):
                        nc.tensor.matmul(
                            psc,
                            qT6[:, 2 * i:2 * i + 2, m * P:(m + 1) * P],
                            sT6[:, 2 * i:2 * i + 2, w * NW:(w + 1) * NW],
                            start=(i == 0),
                            stop=False,
                            perf_mode=DR,
                        )
                    nc.tensor.matmul(
                        psc,
                        qT6[:, 4, m * P:(m + 1) * P],
                        sT6[:, 4, w * NW:(w + 1) * NW],
                        start=False,
                        stop=True,
                    )
                    nc.vector.max(mxs[m][:, w * 8:(w + 1) * 8], psc)
                    # pass-4 finale: tiny DVE reduce folds the first-k-of-8
                    # sum per way -> [128, 5]; the 19-column indicator
                    # matmuls run in batches of 5 (covering completed
                    # m-tiles only) so the PE pays the qT<->rx stationary
                    # swap bubble twice per batch instead of twice per unit
                    if w == WAY - 1:
                        rx = mxp.tile([P, WAY], bf16, tag="rx",
                                      name=f"rx{m}")
                        rxs[m] = rx
                        with nc.allow_low_precision(
                                reason="sum of k<=8 bf16 topk values"):
                            nc.vector.tensor_reduce(
                                rx,
                                mxs[m].rearrange(
                                    "p (w j) -> p w j", w=WAY)[:, :, :k],
                                axis=mybir.AxisListType.X,
                                op=mybir.AluOpType.add,
                            )
                        if m in (5, 10, 14):
                            for mm in range(m - 5 if m < 14 else 10, m):
                                nc.tensor.matmul(
                                    out_ps, rxs[mm],
                                    ind_sb[:, mm * NQ:(mm + 1) * NQ],
                                    start=(mm == 0), stop=False)
            nc.tensor.matmul(
                out_ps, rxs[MT - 1], ind_sb[:, (MT - 1) * NQ:MT * NQ],
                start=False, stop=True)

            # ---- epilogue: copy [5, 19] out of PSUM, DMA out ----
            out_k = const.tile([WAY, NQ], f32, name="out_k")
            nc.vector.tensor_copy(out_k, out_ps)
            nc.sync.dma_start(out=out_d, in_=out_k)

    nc.compile()
    return nc


def get_program(k: int):
    if k not in _prog_cache:
        _prog_cache[k] = _build(k)
    return _prog_cache[k]


def make_in_maps(input1: np.ndarray, input2: np.ndarray, k: int):
    """Shard full inputs into per-core input maps.

    Prep done here (host side of the shard/replicate step): fp8 cast +
    chunk-band transpose of q; L2-normalize+scale+cast+transpose of the
    replicated support features; indicator matrix with the per-patch-row
    1/(k*16*|q_p|) factor folded in.
    """
    input1 = np.asarray(input1, dtype=np.float32)
    input2 = np.asarray(input2, dtype=np.float32)
    in_maps = []
    for core in range(N_CORES):
        b = core // 4
        qs = (core % 4) * NQ
        qe = min(Q, qs + NQ)
        nq = qe - qs
        qdat = input1[b].reshape(Q, HW, C)[qs:qe].reshape(-1, C)
        qfull = np.ones((PAD_P, C), np.float32)
        qfull[: nq * HW] = qdat
        # qT in 5 chunk bands of [128, 1920] fp8 (raw: no normalization)
        qTf = qfull.T.astype(FP8)  # [640, 1920]
        qT = np.ascontiguousarray(
            qTf.reshape(KC, P, PAD_P).transpose(1, 0, 2).reshape(
                P, KC * PAD_P))
        # support: normalize, scale x16 into fp8 range, transpose to bands
        sfull = np.ones((PAD_S, C), np.float32)
        sfull[:NS] = input2[b].reshape(NS, C)
        s_n = (16.0 * sfull / np.linalg.norm(sfull, axis=1, keepdims=True)
               ).astype(FP8)
        sTf = s_n.T  # [640, 2560]
        sT = np.ascontiguousarray(
            sTf.reshape(KC, P, PAD_S).transpose(1, 0, 2).reshape(
                P, KC * PAD_S))
        # indicator: patch row p of M-tile t belongs to query (t*128+p)//HW,
        # pre-scaled by 1/(k * 16 * |q_row|) (fp8-consistent norms)
        qn = np.linalg.norm(qfull.astype(FP8).astype(np.float32), axis=1)
        ind = np.zeros((P, MT * NQ), np.float32)
        g = np.arange(MT * P)
        j = g // HW
        valid = j < nq
        ind[g[valid] % P, (g[valid] // P) * NQ + j[valid]] = (
            1.0 / (k * 16.0 * qn[g[valid]]))
        in_maps.append({
            "qT": qT, "sT": sT,
            "ind": ind.astype(ml_dtypes.bfloat16),
        })
    return in_maps


def gather_out(results) -> np.ndarray:
    out = np.zeros((B, Q, WAY), np.float32)
    for core in range(N_CORES):
        b = core // 4
        qs = (core % 4) * NQ
        n = min(Q, qs + NQ) - qs
        out[b, qs:qs + n] = results[core]["out"].T[:n]
    return out


def kernel(input1, input2, neighbor_k):
    k = int(np.asarray(neighbor_k))
    nc = get_program(k)
    in_maps = make_in_maps(input1, input2, k)
    # the axon-tunneled device occasionally reports a transient
    # "unrecoverable" state right after a previous process's teardown;
    # it recovers within seconds, so retry a couple of times
    import time
    last = None
    for attempt in range(3):
        try:
            res = run_bass_kernel_spmd(
                nc, in_maps, core_ids=list(range(N_CORES)))
            return gather_out(res.results)
        except Exception as e:  # noqa: BLE001
            last = e
            if attempt < 2:
                time.sleep(20.0 * (attempt + 1))
    raise last


# revision 47
# speedup vs baseline: 1.0284x; 1.0284x over previous
"""TRN2 Bass kernel for nn_MetaBaseline (DN4-style local-descriptor kNN).

Reference computation (per batch b):
  q = normalize(input1[b].reshape(75*100, 640), axis=-1)       # query patches
  s = normalize(input2[b].reshape(2500, 640), axis=-1)         # support descs
  scores = q @ s.T                                             # [7500, 2500]
  per way group w (columns [500w, 500w+500)): top-k per row, mean over k,
  then sum over the 100 patches of each query -> out [75, 5].

Sharding: data-parallel over (b, query-quarter): 8 cores, each handles one
batch's quarter of queries (19 queries padded) with that batch's full
support replicated (per the sharding hint).

V3 architecture (fp8, balanced PE/DVE at ~50us each):
- Shard-time input prep (host, part of the sharding/replication step):
  support features are L2-normalized, scaled by 16 (fp8 e4m3 dynamic range),
  cast to fp8 and laid out pre-transposed in 5 chunk bands; queries are cast
  to fp8 and pre-transposed WITHOUT normalization - a positive per-row scale
  cannot change that row's top-k, so 1/(k*16*|q_p|) is folded into the
  host-built indicator matrix that the device uses for the final per-query
  summation. The dominant compute - the 3.07 GMAC/core similarity matmul,
  the 4.7M-element/core top-k scan, and all reductions - runs on device.
- Scores: per (m-tile, way): 2 DoubleRow fp8 matmuls (chunk pairs 01, 23)
  + 1 plain fp8 matmul (chunk 4) accumulate [128, 500] into one PSUM bank.
  (A padded 3rd DoubleRow pair measured strictly slower on HW: DR streams
  both k-tiles' columns, so the zero band costs real feed cycles.)
- Top-k: DVE max8 straight from PSUM -> bf16 mxs [128, 5*8] per m-tile.
  max8 has no DVE fast modes (and measures ~600ns from SBUF or PSUM
  alike), so the 75 x 500-element scan (~46us) is the hard DVE floor;
  the PE's 3-matmul unit (~650ns) makes the loop mutually saturated.
- Finale per m-tile (pass 4): bf16 matmul of the pre-scaled indicator
  with mxs accumulates [19, 40] in PSUM across m-tiles; epilogue reduces
  the first k of each 8 directly from PSUM and DMAs out [19, 5] fp32.
- DMA choreography: only the pass-0 critical set (~0.8MB) transfers
  immediately; the rest is issued later from idle engines (behind dummy
  delay ops) so it does not steal HBM bandwidth from the critical set.
"""
import os
from contextlib import ExitStack

import numpy as np
import ml_dtypes

import concourse.bass as bass  # noqa: F401
import concourse.mybir as mybir
import concourse.tile as tile
from concourse import bacc
from concourse.bass_utils import run_bass_kernel_spmd

# Problem geometry (hardcoded per contest rules)
B, Q, WAY, SHOT, H, W, C = 2, 75, 5, 5, 10, 10, 640
HW = H * W               # 100 patches per query / support image
NQ = 19                  # queries per core (4 cores x 19 = 76 >= 75)
MT = 15                  # patch M-tiles of 128 -> 1920 rows (1900 real)
PAD_P = MT * 128
NS = WAY * SHOT * HW     # 2500 support descriptors per batch
PAD_S = 2560             # padded support count (20 tiles of 128)
KC = 5                   # C chunks of 128 (640 = 5*128)
P = 128
NW = SHOT * HW           # 500 support descriptors per way group
N_CORES = 8
N_WARM = int(os.environ.get("N_WARM", "34"))

FP8 = ml_dtypes.float8_e4m3

_prog_cache: dict[int, object] = {}


def _build(k: int):
    """Build + compile the per-core SPMD program for neighbor_k == k."""
    assert 1 <= k <= 8, f"neighbor_k={k} not supported (need 1..8)"
    nc = bacc.Bacc("TRN2", target_bir_lowering=False, debug=False)
    f32 = mybir.dt.float32
    bf16 = mybir.dt.bfloat16
    fp8 = mybir.dt.float8e4
    DR = mybir.MatmulPerfMode.DoubleRow

    qT_d = nc.dram_tensor("qT", [P, KC * PAD_P], fp8, kind="ExternalInput").ap()
    sT_d = nc.dram_tensor("sT", [P, KC * PAD_S], fp8, kind="ExternalInput").ap()
    ind_d = nc.dram_tensor("ind", [P, MT * NQ], bf16, kind="ExternalInput").ap()
    # output is [way, query] (transposed): the finale matmul uses the tiny
    # [128, 5] reduced-topk tile as the stationary operand (cheap LDW) and
    # the indicator as the moving one; host gather transposes back
    out_d = nc.dram_tensor("out", [WAY, NQ], f32, kind="ExternalOutput").ap()

    with tile.TileContext(nc) as tc:
        with ExitStack() as ctx:
            const = ctx.enter_context(tc.tile_pool(name="const", bufs=1))
            big = ctx.enter_context(tc.tile_pool(name="big", bufs=1))
            mxp = ctx.enter_context(tc.tile_pool(name="mxp", bufs=MT))
            outp = ctx.enter_context(
                tc.tile_pool(name="outp", bufs=1, space="PSUM")
            )
            spp = ctx.enter_context(
                tc.tile_pool(name="spp", bufs=7, space="PSUM")
            )

            qT = big.tile([P, KC * PAD_P], fp8, name="qT")     # chunk bands
            sT = big.tile([P, KC * PAD_S], fp8, name="sT")     # chunk bands
            ind_sb = const.tile([P, MT * NQ], bf16, name="ind_sb")
            qT6 = qT.rearrange("p (c n) -> p c n", c=KC)
            sT6 = sT.rearrange("p (c n) -> p c n", c=KC)

            out_ps = outp.tile([WAY, NQ], f32)

            # ---- PE warmup (HAM clock ramp; no DMA deps) ----
            wtile = const.tile([P, P], fp8, name="wtile")
            nc.vector.memset(wtile, 1.0)
            for i in range(N_WARM):
                wps = spp.tile([P, NW], f32, tag="psc", name=f"w{i}")
                nc.tensor.matmul(wps[:, 0:P], wtile, wtile,
                                 start=True, stop=True)

            # ---- DMAs, issue-parallel across idle engines ----
            # The first (m,w) unit needs qT m-tiles 0-4 (all bands) and sT
            # way 0 only (~0.8MB); everything else streams under pass 0+.
            # ACT: qT in three m-tile thirds; Sync: sT way-slices in pass
            # order; GpSimd: indicator (needed only in pass 4).
            sT_dv = sT_d.rearrange("p (c n) -> p c n", c=KC)
            qT_dv = qT_d.rearrange("p (c n) -> p c n", c=KC)
            # immediate critical set (first unit needs sT way 0 + qT m0):
            # sT way-0 bands split sync/ACT, qT m-tiles 0-4 split ACT/GpSimd
            # immediate critical set (first unit needs sT way 0 + qT m0):
            # sT way-0 bands on Sync, qT m-tiles 0-4 split ACT/GpSimd
            nc.sync.dma_start(out=sT6[:, :, 0:NW], in_=sT_dv[:, :, 0:NW])
            MG = 5 * P  # m-tile third (5 tiles of 128 cols)
            HG = MG // 2
            nc.scalar.dma_start(out=qT6[:, :, 0:HG], in_=qT_dv[:, :, 0:HG])
            nc.gpsimd.dma_start(out=qT6[:, :, HG:MG], in_=qT_dv[:, :, HG:MG])
            # deferred behind idle-engine delay ops so their descriptors
            # don't compete with the critical set:
            # ACT: qT thirds 1-2 after dummy copies
            acts = const.tile([P, P], f32, name="acts")
            for g in (1, 2):
                for _ in range(14 if g == 1 else 6):
                    nc.scalar.copy(acts, wtile)
                nc.scalar.dma_start(
                    out=qT6[:, :, g * MG:(g + 1) * MG],
                    in_=qT_dv[:, :, g * MG:(g + 1) * MG])
            # GpSimd: sT ways 1-4 + indicator, spaced by memsets
            gscr = const.tile([P, 2048], fp8, name="gscr")
            for w in (1, 2, 3, 4):
                for _ in range(3 if w == 1 else 2):
                    nc.gpsimd.memset(gscr, 0.0)
                nc.gpsimd.dma_start(
                    out=sT6[:, :, w * NW:(w + 1) * NW],
                    in_=sT_dv[:, :, w * NW:(w + 1) * NW])
            nc.gpsimd.dma_start(out=ind_sb, in_=ind_d)

            # ---- main loop: way-outer, m-tile inner ----
            mxs = [None] * MT
            rxs = [None] * MT
            for w in range(WAY):
                for m in range(MT):
                    if w == 0:
                        mxs[m] = mxp.tile([P, WAY * 8], bf16, tag="mx",
                                          name=f"mx{m}")
                    psc = spp.tile([P, NW], f32, tag="psc",
                                   name=f"psc{m}_{w}")
                    for i in range(2):
                        nc.tensor.matmul(
                            psc,
                            qT6[:, 2 * i:2 * i + 2, m * P:(m + 1) * P],
                            sT6[:, 2 * i:2 * i + 2, w * NW:(w + 1) * NW],
                            start=(i == 0),
                            stop=False,
                            perf_mode=DR,
                        )
                    nc.tensor.matmul(
                        psc,
                        qT6[:, 4, m * P:(m + 1) * P],
                        sT6[:, 4, w * NW:(w + 1) * NW],
                        start=False,
                        stop=True,
                    )
                    nc.vector.max(mxs[m][:, w * 8:(w + 1) * 8], psc)
                    # pass-4 finale: tiny DVE reduce folds the first-k-of-8
                    # sum per way -> [128, 5], then a 5-column indicator
                    # matmul (delayed by one m-tile so the PE never waits
                    # on the just-issued max8) accumulates [19, 5] in PSUM
                    if w == WAY - 1:
                        rx = mxp.tile([P, WAY], bf16, tag="rx",
                                      name=f"rx{m}")
                        rxs[m] = rx
                        with nc.allow_low_precision(
                                reason="sum of k<=8 bf16 topk values"):
                            nc.vector.tensor_reduce(
                                rx,
                                mxs[m].rearrange(
                                    "p (w j) -> p w j", w=WAY)[:, :, :k],
                                axis=mybir.AxisListType.X,
                                op=mybir.AluOpType.add,
                            )
                        if m > 0:
                            nc.tensor.matmul(
                                out_ps,
                                rxs[m - 1], ind_sb[:, (m - 1) * NQ:m * NQ],
                                start=(m == 1), stop=False)
            nc.tensor.matmul(
                out_ps, rxs[MT - 1], ind_sb[:, (MT - 1) * NQ:MT * NQ],
                start=False, stop=True)

            # ---- epilogue: copy [5, 19] out of PSUM, DMA out ----
            out_k = const.tile([WAY, NQ], f32, name="out_k")
            nc.vector.tensor_copy(out_k, out_ps)
            nc.sync.dma_start(out=out_d, in_=out_k)

    nc.compile()
    return nc


def get_program(k: int):
    if k not in _prog_cache:
        _prog_cache[k] = _build(k)
    return _prog_cache[k]


def make_in_maps(input1: np.ndarray, input2: np.ndarray, k: int):
    """Shard full inputs into per-core input maps.

    Prep done here (host side of the shard/replicate step): fp8 cast +
    chunk-band transpose of q; L2-normalize+scale+cast+transpose of the
    replicated support features; indicator matrix with the per-patch-row
    1/(k*16*|q_p|) factor folded in.
    """
    input1 = np.asarray(input1, dtype=np.float32)
    input2 = np.asarray(input2, dtype=np.float32)
    in_maps = []
    for core in range(N_CORES):
        b = core // 4
        qs = (core % 4) * NQ
        qe = min(Q, qs + NQ)
        nq = qe - qs
        qdat = input1[b].reshape(Q, HW, C)[qs:qe].reshape(-1, C)
        qfull = np.ones((PAD_P, C), np.float32)
        qfull[: nq * HW] = qdat
        # qT in 5 chunk bands of [128, 1920] fp8 (raw: no normalization)
        qTf = qfull.T.astype(FP8)  # [640, 1920]
        qT = np.ascontiguousarray(
            qTf.reshape(KC, P, PAD_P).transpose(1, 0, 2).reshape(
                P, KC * PAD_P))
        # support: normalize, scale x16 into fp8 range, transpose to bands
        sfull = np.ones((PAD_S, C), np.float32)
        sfull[:NS] = input2[b].reshape(NS, C)
        s_n = (16.0 * sfull / np.linalg.norm(sfull, axis=1, keepdims=True)
               ).astype(FP8)
        sTf = s_n.T  # [640, 2560]
        sT = np.ascontiguousarray(
            sTf.reshape(KC, P, PAD_S).transpose(1, 0, 2).reshape(
                P, KC * PAD_S))
        # indicator: patch row p of M-tile t belongs to query (t*128+p)//HW,
        # pre-scaled by 1/(k * 16 * |q_row|) (fp8-consistent norms)
        qn = np.linalg.norm(qfull.astype(FP8).astype(np.float32), axis=1)
        ind = np.zeros((P, MT * NQ), np.float32)
        g = np.arange(MT * P)
        j = g // HW
        valid = j < nq
        ind[g[valid] % P, (g[valid] // P) * NQ + j[valid]] = (
            1.0 / (k * 16.0 * qn[g[valid]]))
        in_maps.append({
            "qT": qT, "sT": sT,
            "ind": ind.astype(ml_dtypes.bfloat16),
        })
    return in_maps


def gather_out(results) -> np.ndarray:
    out = np.zeros((B, Q, WAY), np.float32)
    for core in range(N_CORES):
        b = core // 4
        qs = (core % 4) * NQ
        n = min(Q, qs + NQ) - qs
        out[b, qs:qs + n] = results[core]["out"].T[:n]
    return out


def kernel(input1, input2, neighbor_k):
    k = int(np.asarray(neighbor_k))
    nc = get_program(k)
    in_maps = make_in_maps(input1, input2, k)
    # the axon-tunneled device occasionally reports a transient
    # "unrecoverable" state right after a previous process's teardown;
    # it recovers within seconds, so retry a couple of times
    import time
    last = None
    for attempt in range(3):
        try:
            res = run_bass_kernel_spmd(
                nc, in_maps, core_ids=list(range(N_CORES)))
            return gather_out(res.results)
        except Exception as e:  # noqa: BLE001
            last = e
            if attempt < 2:
                time.sleep(20.0 * (attempt + 1))
    raise last
